# revision 1
# baseline (speedup 1.0000x reference)
"""BilinearRelationNet Trainium2 kernel (8 NeuronCores, data-parallel over batch).

Layout strategy (feature-on-partitions, batch-on-free-dim):
  phase 1 (per tower t, per 512-row chunk): SWDGE casting DMA streams x fp32 ->
    fp16 with partition p holding 4 CONTIGUOUS rows (8KB contiguous reads);
    TensorE transposes 128x128 blocks (fp16 PSUM); DVE evacuates; TensorE mm1
    (x@W1) accumulates fp32 PSUM; ACT evacuates h_pre to fp16 SBUF; DVE
    bn_stats on the fp16 copy.
  ONE AllGather for both towers' (mean, E[h^2]) — the runtime barriers the
    chip around each collective, so a single one minimizes dead time (remote
    DMA data transfers do not work cross-core in this environment).
  BN fold: relu(s*h + b) = s*relu(h + b/s) for s>0, so the BN scale folds into
    W2 (rows scaled on-chip after stats) and only a bias-add+relu remains on
    the h_pre pass (one engine op per slab).
  phase B (per tower): bias+relu -> mm2 (scaled W2) -> relu+b2 via activation
    bias on evacuation (no bias matmuls anywhere).
  phase C: elementwise combine (ACT/DVE/GPS) -> mm3 against W3 extended with
    ones columns (dot/n1/n2 row-sums ride along in the same PSUM tile) ->
    relu+b3 -> mm4 -> stage per-row scalars.
  phase D: gather per-row scalars into [128, R/128] tiles, cosine+sigmoid
    finalization, DMA out.

All small constants ride in two packed DRAM tensors (one f16, one f32) so
startup is two DMAs. The chunk DMA permutes rows (free position nb*128+p
holds row 4p+nb); the host applies the inverse permutation after gathering.
b1 is dropped: BN subtracts the batch mean of (x@W1 + b1), so b1 cancels.
"""

import sys

sys.path.insert(0, "/opt/trn_rl_repo")

import numpy as np
import concourse.bass as bass
import concourse.bacc as bacc
import concourse.tile as tile
import concourse.mybir as mybir
from concourse import bass_utils

F32 = mybir.dt.float32
F16 = mybir.dt.float16
AF = mybir.ActivationFunctionType
ALU = mybir.AluOpType

N_CORES = 8
D = 512
H = 256
BN_EPS = 1e-5

# packed f16 const layout (columns)
W1_OFF = 0            # 4 chunks x 256 = 1024
W2_OFF = 1024         # 2 chunks x 128 = 256
W3_OFF = 1280         # 5 chunks x 67 = 335
IDEN_OFF = 1615       # 128
W4_OFF = 1743         # 1 (partitions 0:64)
F16_COLS = 1744
# packed f32 const layout (columns)
B2_OFF, GAM_OFF, BBN_OFF, ALP_OFF, BET_OFF, B4_OFF, B3_OFF, ZERO_OFF = (
    0, 1, 3, 5, 6, 7, 8, 9,
)
F32_COLS = 10


def build_nc(n_chunks: int):
    """One SPMD program; each core handles R = n_chunks*512 rows of both x1/x2."""
    R = n_chunks * 512
    nc = bacc.Bacc("TRN2", target_bir_lowering=False, debug=False, num_devices=N_CORES)

    x_dram = [
        nc.dram_tensor("x1", [R, D], F32, kind="ExternalInput"),
        nc.dram_tensor("x2", [R, D], F32, kind="ExternalInput"),
    ]
    c16_d = nc.dram_tensor("c16", [128, F16_COLS], F16, kind="ExternalInput")
    c32_d = nc.dram_tensor("c32", [128, F32_COLS], F32, kind="ExternalInput")
    out_d = nc.dram_tensor("out", [R], F32, kind="ExternalOutput")

    with tile.TileContext(nc) as tc:
        with (
            tc.tile_pool(name="const", bufs=1) as cpool,
            tc.tile_pool(name="persist", bufs=1) as hpool,
            tc.tile_pool(name="dram", bufs=1, space="DRAM") as dpool,
        ):
            # ---- constants to SBUF: two packed DMAs ----
            c16 = cpool.tile([128, F16_COLS], F16, tag="c16")
            nc.sync.dma_start(c16[:], c16_d[:])
            c32 = cpool.tile([128, F32_COLS], F32, tag="c32")
            nc.sync.dma_start(c32[:], c32_d[:])
            w1s = c16[:, W1_OFF : W1_OFF + 1024]
            w2s = c16[:, W2_OFF : W2_OFF + 256]
            w3s = c16[:, W3_OFF : W3_OFF + 335]
            idens = c16[:, IDEN_OFF : IDEN_OFF + 128]
            w4s = c16[0:64, W4_OFF : W4_OFF + 1]
            b2s = c32[:, B2_OFF : B2_OFF + 1]
            gammas = c32[:, GAM_OFF : GAM_OFF + 2]
            betabns = c32[:, BBN_OFF : BBN_OFF + 2]
            alphas = c32[:, ALP_OFF : ALP_OFF + 1]
            betas = c32[:, BET_OFF : BET_OFF + 1]
            b4s = c32[:, B4_OFF : B4_OFF + 1]
            b3s = c32[0:64, B3_OFF : B3_OFF + 1]
            zeros1 = c32[:, ZERO_OFF : ZERO_OFF + 1]

            # ---- persistent buffers ----
            hp = [
                [hpool.tile([128, R], F16, tag=f"hp{t}{m}", name=f"hp{t}{m}") for m in range(2)]
                for t in range(2)
            ]
            hh = [hpool.tile([128, R], F16, tag=f"hh{t}", name=f"hh{t}") for t in range(2)]
            sbstats = [
                hpool.tile([128, 6 * n_chunks], F32, tag=f"bst{s}", name=f"bst{s}") for s in range(4)
            ]
            ncols = R // 128
            tq = [hpool.tile([128, ncols], F32, tag=f"tq{q}", name=f"tq{q}") for q in range(4)]
            w2t = [hpool.tile([128, 256], F16, tag=f"w2t{t}", name=f"w2t{t}") for t in range(2)]
            bp = [hpool.tile([128, 2], F32, tag=f"bp{t}", name=f"bp{t}") for t in range(2)]
            arin = hpool.tile([128, 8], F32, tag="arin")
            agout = hpool.tile([128, 64], F32, tag="agout")
            stw = hpool.tile([128, 16], F32, tag="stw")
            st2 = hpool.tile([128, 16], F32, tag="st2")

            # ================= phase 1 + stats =================
            with (
                tc.tile_pool(name="p1sb", bufs=3) as p1,
                tc.tile_pool(name="p1ps", bufs=2, space="PSUM") as pp1,
            ):
                for t in range(2):
                    for c in range(n_chunks):
                        rows = slice(c * 512, (c + 1) * 512)
                        # partition p <- rows 4p..4p+3 (8KB contiguous per
                        # partition); free position nb*128+p <-> row 4p+nb
                        xfs = p1.tile([128, 4 * 512], F16, tag="xfs")
                        nc.gpsimd.dma_start(
                            xfs.rearrange("p (nb d) -> p nb d", nb=4),
                            x_dram[t][rows, :].rearrange("(p nb) d -> p nb d", nb=4),
                        )
                        xT = p1.tile([128, 4 * 512], F16, tag="xT")
                        for half in range(2):
                            ptr = pp1.tile([128, 1024], F16, tag="ptr", bufs=2)
                            for dci in range(2):
                                dc = half * 2 + dci
                                for nb in range(4):
                                    nc.tensor.transpose(
                                        ptr[:, dci * 512 + nb * 128 : dci * 512 + (nb + 1) * 128],
                                        xfs[:, nb * 512 + dc * 128 : nb * 512 + (dc + 1) * 128],
                                        idens,
                                    )
                            nc.vector.tensor_copy(
                                xT[:, half * 1024 : (half + 1) * 1024], ptr[:]
                            )
                        acc = pp1.tile([128, 1024], F32, tag="acc", bufs=3)
                        for m in range(2):
                            for dc in range(4):
                                nc.tensor.matmul(
                                    acc[:, m * 512 : (m + 1) * 512],
                                    w1s[:, dc * H + m * 128 : dc * H + (m + 1) * 128],
                                    xT[:, dc * 512 : (dc + 1) * 512],
                                    start=(dc == 0),
                                    stop=(dc == 3),
                                )
                        for m in range(2):
                            dst = hp[t][m][:, c * 512 : (c + 1) * 512]
                            nc.scalar.activation(
                                dst, acc[:, m * 512 : (m + 1) * 512], AF.Copy
                            )
                            nc.vector.bn_stats(
                                sbstats[t * 2 + m][:, c * 6 : (c + 1) * 6],
                                acc[:, m * 512 : (m + 1) * 512],
                            )

                    # local stats for tower t -> arin cols 4t..4t+3
                    for m in range(2):
                        s = t * 2 + m
                        aggr = stw[:, s * 2 : s * 2 + 2]
                        nc.vector.bn_aggr(aggr, sbstats[s][:])
                        nc.vector.tensor_copy(
                            arin[:, 4 * t + 2 * m : 4 * t + 2 * m + 1], aggr[:, 0:1]
                        )
                        msq = stw[:, 8 + s : 9 + s]
                        nc.vector.tensor_tensor(msq, aggr[:, 0:1], aggr[:, 0:1], ALU.mult)
                        nc.vector.tensor_tensor(
                            arin[:, 4 * t + 2 * m + 1 : 4 * t + 2 * m + 2],
                            aggr[:, 1:2], msq, ALU.add,
                        )

            # ---- single AllGather + phases B/C (phase-1 pools closed) ----
            with (
                tc.tile_pool(name="p2sb", bufs=3) as p2,
                tc.tile_pool(name="p3sb", bufs=3) as p3,
                tc.tile_pool(name="p2ps", bufs=2, space="PSUM") as pp2,
                tc.tile_pool(name="p3ps", bufs=2, space="PSUM") as pp3,
            ):
                bnc_in = dpool.tile([128, 8], F32, name="bnc_in")
                bnc_out = dpool.tile(
                    [128 * N_CORES, 8], F32, addr_space="Shared", name="bnc_out"
                )
                nc.sync.dma_start(bnc_in[:], arin[:])
                nc.gpsimd.collective_compute(
                    "AllGather",
                    ALU.bypass,
                    ins=[bnc_in.opt()],
                    outs=[bnc_out.opt()],
                    replica_groups=[list(range(N_CORES))],
                )
                nc.sync.dma_start(
                    agout.rearrange("p (r j) -> p r j", r=N_CORES),
                    bnc_out.rearrange("(r p) j -> p r j", p=128),
                )
                # tree-sum 8 rank blocks of 8 cols -> sums [128, 8]
                nc.vector.tensor_tensor(
                    st2[:, 0:16], agout[:, 0:16], agout[:, 16:32], ALU.add
                )
                nc.vector.tensor_tensor(
                    stw[:, 0:16], agout[:, 32:48], agout[:, 48:64], ALU.add
                )
                nc.vector.tensor_tensor(st2[:, 0:16], st2[:, 0:16], stw[:, 0:16], ALU.add)
                sums = stw[:, 0:8]
                nc.vector.tensor_tensor(sums, st2[:, 0:8], st2[:, 8:16], ALU.add)
                # cols of sums: [t*4 + 2m]=sum(mean), [t*4+2m+1]=sum(Eh2)
                gm4 = st2[:, 0:4]   # means for sets (t,m) order [00,01,10,11]
                nc.vector.tensor_scalar(gm4, sums[:, 0:8:2], 1.0 / N_CORES, None, ALU.mult)
                gE4 = st2[:, 4:8]
                nc.vector.tensor_scalar(gE4, sums[:, 1:8:2], 1.0 / N_CORES, None, ALU.mult)
                gmsq = st2[:, 8:12]
                nc.vector.tensor_tensor(gmsq, gm4, gm4, ALU.mult)
                var4 = st2[:, 12:16]
                nc.vector.tensor_tensor(var4, gE4, gmsq, ALU.subtract)
                vare4 = stw[:, 0:4]
                nc.vector.tensor_scalar(vare4, var4, float(BN_EPS), None, ALU.add)
                std4 = stw[:, 4:8]
                nc.scalar.activation(std4, vare4, AF.Sqrt)
                istd4 = stw[:, 8:12]
                nc.vector.reciprocal(istd4, std4)
                gam4 = stw[:, 12:16]
                nc.vector.tensor_copy(gam4[:, 0:2], gammas)
                nc.vector.tensor_copy(gam4[:, 2:4], gammas)
                scale4 = stw[:, 0:4]
                nc.vector.tensor_tensor(scale4, istd4, gam4, ALU.mult)
                # bias'4 = betabn/scale - gm
                rsc4 = stw[:, 4:8]
                nc.vector.reciprocal(rsc4, scale4)
                bet4 = stw[:, 8:12]
                nc.vector.tensor_copy(bet4[:, 0:2], betabns)
                nc.vector.tensor_copy(bet4[:, 2:4], betabns)
                bb4 = stw[:, 12:16]
                nc.vector.tensor_tensor(bb4, bet4, rsc4, ALU.mult)
                bp4 = st2[:, 8:12]
                nc.vector.tensor_tensor(bp4, bb4, gm4, ALU.subtract)
                for t in range(2):
                    nc.vector.tensor_copy(bp[t][:], bp4[:, 2 * t : 2 * t + 2])
                    for m in range(2):
                        nc.vector.tensor_scalar(
                            w2t[t][:, m * 128 : (m + 1) * 128],
                            w2s[:, m * 128 : (m + 1) * 128],
                            scale4[:, 2 * t + m : 2 * t + m + 1],
                            None, ALU.mult,
                        )

                # ====== phases B + C interleaved per 1024-col block ======
                CW = 1024

                def emit_B(t, c):
                    cols = slice(c * CW, (c + 1) * CW)
                    hn0 = p2.tile([128, CW], F16, tag=f"hn0_{t}")
                    nc.scalar.activation(
                        hn0, hp[t][0][:, cols], AF.Relu, bias=bp[t][:, 0:1]
                    )
                    hn1 = p2.tile([128, CW], F16, tag=f"hn1_{t}")
                    nc.vector.tensor_scalar(
                        hn1, hp[t][1][:, cols], bp[t][:, 1:2], zeros1,
                        ALU.add, ALU.max,
                    )
                    for hf in range(2):
                        hs = slice(hf * 512, (hf + 1) * 512)
                        pw = pp2.tile([128, 512], F32, tag="pw")
                        nc.tensor.matmul(
                            pw[:], w2t[t][:, 0:128], hn0[:, hs], start=True, stop=False
                        )
                        nc.tensor.matmul(
                            pw[:], w2t[t][:, 128:256], hn1[:, hs], start=False, stop=True
                        )
                        dst = hh[t][:, c * CW + hf * 512 : c * CW + (hf + 1) * 512]
                        if hf == 0:
                            nc.scalar.activation(dst, pw[:], AF.Relu, bias=b2s)
                        else:
                            nc.vector.tensor_scalar(
                                dst, pw[:], b2s, zeros1, ALU.add, ALU.max
                            )

                def emit_C(c):
                    cols = slice(c * CW, (c + 1) * CW)
                    h0 = hh[0][:, cols]
                    h1 = hh[1][:, cols]
                    p_t = p3.tile([128, CW], F16, tag="p_t")
                    nc.vector.tensor_tensor(p_t[:], h0, h1, ALU.mult)
                    dd = p3.tile([128, CW], F16, tag="dd")
                    nc.vector.tensor_tensor(dd[:], h0, h1, ALU.subtract)
                    q_t = p3.tile([128, CW], F16, tag="q_t")
                    nc.scalar.activation(q_t[:], dd[:], AF.Abs)
                    r_t = p3.tile([128, CW], F16, tag="r_t")
                    nc.vector.tensor_tensor(r_t[:], h0, h1, ALU.add)
                    s1_t = p3.tile([128, CW], F16, tag="s1_t")
                    nc.vector.tensor_tensor(s1_t[:], h0, h0, ALU.mult)
                    s2_t = p3.tile([128, CW], F16, tag="s2_t")
                    nc.gpsimd.tensor_tensor(s2_t[:], h1, h1, ALU.mult)

                    stage = p3.tile([33, CW], F32, tag="stage")
                    r64 = p3.tile([64, CW], F16, tag="r64")
                    rhs5 = [p_t, r_t, s1_t, q_t, s2_t]
                    pw3 = pp3.tile([128, CW], F32, tag="pw3")
                    for hf in range(2):
                        hs = slice(hf * 512, (hf + 1) * 512)
                        for k in range(5):
                            nc.tensor.matmul(
                                pw3[0:67, hs],
                                w3s[:, k * 67 : (k + 1) * 67],
                                rhs5[k][:, hs],
                                start=(k == 0),
                                stop=(k == 4),
                            )
                        nc.scalar.activation(
                            r64[:, hs], pw3[0:64, hs], AF.Relu, bias=b3s
                        )
                    nc.scalar.activation(stage[0:3, :], pw3[64:67, :], AF.Copy)
                    pw4 = pp3.tile([1, CW], F32, tag="pw4", bufs=1)
                    for hf in range(2):
                        hs = slice(hf * 512, (hf + 1) * 512)
                        nc.tensor.matmul(
                            pw4[:, hs], w4s, r64[:, hs], start=True, stop=True
                        )
                    nc.vector.tensor_copy(stage[32:33, :], pw4[:])
                    ppc = CW // ncols
                    for q in range(4):
                        sp = q if q < 3 else 32
                        nc.sync.dma_start(
                            tq[q][c * ppc : (c + 1) * ppc, :], stage[sp : sp + 1, :]
                        )

                for c in range(R // CW):
                    emit_B(0, c)
                    emit_B(1, c)
                    emit_C(c)

            # ================= phase D: finalize =================
            fin = hpool.tile([128, 6 * ncols], F32, tag="fin")

            def fcol(i):
                return fin[:, i * ncols : (i + 1) * ncols]

            nc.vector.tensor_tensor(fcol(0), tq[1][:], tq[2][:], ALU.mult)  # n1*n2
            nc.vector.tensor_scalar(fcol(2), fcol(0), 1e-30, None, ALU.add)
            nc.scalar.activation(fcol(1), fcol(2), AF.Sqrt)
            nc.vector.reciprocal(fcol(2), fcol(1))
            nc.vector.tensor_tensor(fcol(0), tq[0][:], fcol(2), ALU.mult)  # s_math
            nc.vector.tensor_scalar(fcol(1), fcol(0), 0.0, 1.0, ALU.max, ALU.min)
            nc.scalar.activation(fcol(3), tq[3][:], AF.Sigmoid, bias=b4s)
            nc.vector.tensor_scalar(fcol(4), fcol(1), alphas, None, ALU.mult)
            nc.vector.tensor_scalar(fcol(5), fcol(3), betas, None, ALU.mult)
            nc.vector.tensor_tensor(fcol(0), fcol(4), fcol(5), ALU.add)
            nc.vector.tensor_scalar(fcol(1), fcol(0), 0.0, 1.0, ALU.max, ALU.min)
            nc.sync.dma_start(
                out_d.ap().rearrange("(p k) -> p k", p=128), fcol(1)
            )

    nc.compile()
    return nc


_NC_CACHE: dict = {}


def _get_nc(n_chunks):
    if n_chunks not in _NC_CACHE:
        _NC_CACHE[n_chunks] = build_nc(n_chunks)
    return _NC_CACHE[n_chunks]


def _prep_weights(W1, gamma, beta_bn, W2, b2, W3, b3, W4, b4, alpha, beta):
    f16 = np.float16
    f32 = np.float32
    W1 = np.asarray(W1, f32)
    W3 = np.asarray(W3, f32)
    w3e = np.zeros((128, 5, 67), f32)
    # chunk order matches rhs5 = [p, r, s1, q, s2]
    w3e[:, 0, 0:64] = W3[0:128]    # p  (h1*h2)
    w3e[:, 1, 0:64] = W3[256:384]  # r  (h1+h2)
    w3e[:, 3, 0:64] = W3[128:256]  # q  (|h1-h2|)
    w3e[:, 0, 64] = 1.0  # dot = ones . (h1*h2)
    w3e[:, 2, 65] = 1.0  # n1  = ones . h1^2
    w3e[:, 4, 66] = 1.0  # n2  = ones . h2^2
    c16 = np.zeros((128, F16_COLS), f16)
    c16[:, W1_OFF : W1_OFF + 1024] = (
        W1.reshape(4, 128, H).transpose(1, 0, 2).reshape(128, 1024).astype(f16)
    )
    c16[:, W2_OFF : W2_OFF + 256] = (
        np.asarray(W2, f32).reshape(2, 128, 128).transpose(1, 0, 2).reshape(128, 256)
    ).astype(f16)
    c16[:, W3_OFF : W3_OFF + 335] = w3e.reshape(128, 335).astype(f16)
    c16[:, IDEN_OFF : IDEN_OFF + 128] = np.eye(128, dtype=f16)
    c16[0:64, W4_OFF] = np.asarray(W4, f32).reshape(-1).astype(f16)
    c32 = np.zeros((128, F32_COLS), f32)
    c32[:, B2_OFF] = np.asarray(b2, f32).reshape(-1)
    c32[:, GAM_OFF : GAM_OFF + 2] = np.asarray(gamma, f32).reshape(2, 128).T
    c32[:, BBN_OFF : BBN_OFF + 2] = np.asarray(beta_bn, f32).reshape(2, 128).T
    c32[:, ALP_OFF] = np.asarray(alpha, f32).reshape(-1)[0]
    c32[:, BET_OFF] = np.asarray(beta, f32).reshape(-1)[0]
    c32[:, B4_OFF] = np.asarray(b4, f32).reshape(-1)[0]
    c32[0:64, B3_OFF] = np.asarray(b3, f32).reshape(-1)
    return {"c16": c16, "c32": c32}


def _row_perm(R):
    """Device free-position <-> source row, per 512-row chunk (see chunk DMA)."""
    j = np.arange(512)
    pos_of_row = (j % 4) * 128 + j // 4  # row -> device position
    return pos_of_row


def run_on_hw(x1, x2, weights, n_chunks, trace=False):
    R = n_chunks * 512
    nc = _get_nc(n_chunks)
    in_maps = []
    for c in range(N_CORES):
        m = {"x1": np.ascontiguousarray(x1[c * R : (c + 1) * R]),
             "x2": np.ascontiguousarray(x2[c * R : (c + 1) * R])}
        m.update(weights)
        in_maps.append(m)
    r = bass_utils.run_bass_kernel_spmd(
        nc, in_maps, core_ids=list(range(N_CORES)), trace=trace
    )
    pos = _row_perm(R)
    outs = []
    for c in range(N_CORES):
        dev = r.results[c]["out"].reshape(n_chunks, 512)
        outs.append(dev[:, pos].reshape(-1))  # out[row] = dev[pos_of_row]
    return np.concatenate(outs), r


def kernel(x1, x2, W1, b1, gamma, beta_bn, W2, b2, W3, b3, W4, b4, alpha, beta):
    x1 = np.asarray(x1, np.float32)
    x2 = np.asarray(x2, np.float32)
    n_chunks = x1.shape[0] // (N_CORES * 512)
    weights = _prep_weights(W1, gamma, beta_bn, W2, b2, W3, b3, W4, b4, alpha, beta)
    out, _ = run_on_hw(x1, x2, weights, n_chunks)
    return out.astype(np.float32)



# revision 9
# speedup vs baseline: 1.3837x; 1.3837x over previous
"""BilinearRelationNet Trainium2 kernel (8 NeuronCores, data-parallel over batch).

v2 layout strategy (feature-on-partitions, batch-on-free-dim), no device transposes:
  Host pre-transposes and fp16-casts x: xt = x.T [512, R] per core, so DMA loads
    land with the contraction dim (d) on partitions directly. This removes all
    512 TensorE transposes and their PSUM->SBUF evacuations, and halves HBM
    read traffic vs fp32.
  Phase 1 (per tower, per 1024-row group): one HWDGE DMA brings [512, 1024]
    fp16; 16 matmuls (2 h-halves x 4 d-chunks x 2 col-halves) accumulate fp32
    PSUM; ACT/DVE evacuate h_pre to fp16 SBUF; DVE bn_stats on the fp16 copy.
  TWO AllGathers (one per tower), each triggered the moment its tower's local
    stats are ready: AG(tower1) overlaps tower2's phase-1 matmuls; AG(tower2)
    overlaps phase B of tower 1 + the s1=h1^2 precompute.
  BN fold: relu(s*h + b) = s*relu(h + b/s) for s>0 -> scale folds into W2,
    bias+relu fused into one DVE tensor_scalar per slab.
  Phase B (per tower): bias+relu -> mm2 (scaled W2) -> relu+b2 on evacuation.
  Phase C: elementwise combine; mm3 against W3 extended with ones columns so
    dot/n1/n2 row-sums ride in PSUM rows 64:67; ONE fused Relu+bias evacuation
    of rows 0:67 (dot/n1/n2 are nonnegative, so Relu is harmless); mm4; per-row
    scalars DMA'd into [128, R/128] tq tiles (no row permutation needed).
  Phase D: cosine + sigmoid finalization, DMA out.

ACT table-set discipline: sqrt set preloaded at start (Relu/Copy/Abs are filler
in every set); single Sigmoid switch at the tail. b1 is dropped: BN subtracts
the batch mean of (x@W1 + b1), so b1 cancels.
"""

import sys

sys.path.insert(0, "/opt/trn_rl_repo")

import numpy as np
import concourse.bass as bass
import concourse.bacc as bacc
import concourse.tile as tile
import concourse.mybir as mybir
from concourse import bass_utils

F32 = mybir.dt.float32
F16 = mybir.dt.float16
AF = mybir.ActivationFunctionType
ALU = mybir.AluOpType

N_CORES = 8
D = 512
H = 256
BN_EPS = 1e-5

# packed f16 const layout (columns)
W1_OFF = 0            # 4 chunks x 256 = 1024
W2_OFF = 1024         # 2 chunks x 128 = 256
W3_OFF = 1280         # 5 chunks x 67 = 335
W4_OFF = 1615         # 1 (partitions 0:64)
F16_COLS = 1616
# packed f32 const layout (columns)
B2_OFF, GAM_OFF, BBN_OFF, ALP_OFF, BET_OFF, B4_OFF, B3_OFF, ZERO_OFF = (
    0, 1, 3, 5, 6, 7, 8, 9,
)
F32_COLS = 10

G = 1024  # rows per phase-1 group
CW = 1024  # rows per phase-B/C block


def build_nc(n_chunks: int):
    """One SPMD program; each core handles R = n_chunks*512 rows of both x1/x2."""
    R = n_chunks * 512
    assert R % G == 0 and R % CW == 0 and R % 128 == 0
    NG = R // G
    NB = R // CW
    ncols = R // 128
    ppc = CW // ncols
    nc = bacc.Bacc("TRN2", target_bir_lowering=False, debug=False, num_devices=N_CORES)

    xt_dram = [
        nc.dram_tensor("xt1", [D, R], F16, kind="ExternalInput"),
        nc.dram_tensor("xt2", [D, R], F16, kind="ExternalInput"),
    ]
    c16_d = nc.dram_tensor("c16", [128, F16_COLS], F16, kind="ExternalInput")
    c32_d = nc.dram_tensor("c32", [128, F32_COLS], F32, kind="ExternalInput")
    out_d = nc.dram_tensor("out", [R], F32, kind="ExternalOutput")

    with tile.TileContext(nc) as tc:
        with (
            tc.tile_pool(name="const", bufs=1) as cpool,
            tc.tile_pool(name="persist", bufs=1) as hpool,
            tc.tile_pool(name="dram", bufs=1, space="DRAM") as dpool,
        ):
            # ---- constants to SBUF: two packed DMAs ----
            c16 = cpool.tile([128, F16_COLS], F16, tag="c16")
            nc.sync.dma_start(c16[:], c16_d[:])
            c32 = cpool.tile([128, F32_COLS], F32, tag="c32")
            nc.sync.dma_start(c32[:], c32_d[:])
            w1s = c16[:, W1_OFF : W1_OFF + 1024]
            w2s = c16[:, W2_OFF : W2_OFF + 256]
            w3s = c16[:, W3_OFF : W3_OFF + 335]
            w4s = c16[0:64, W4_OFF : W4_OFF + 1]
            b2s = c32[:, B2_OFF : B2_OFF + 1]
            gammas = c32[:, GAM_OFF : GAM_OFF + 2]
            betabns = c32[:, BBN_OFF : BBN_OFF + 2]
            alphas = c32[:, ALP_OFF : ALP_OFF + 1]
            betas = c32[:, BET_OFF : BET_OFF + 1]
            b4s = c32[:, B4_OFF : B4_OFF + 1]
            b3e = c32[0:67, B3_OFF : B3_OFF + 1]
            zeros1 = c32[:, ZERO_OFF : ZERO_OFF + 1]

            # ---- persistent buffers ----
            hp = [
                [hpool.tile([128, R], F16, tag=f"hp{t}{m}", name=f"hp{t}{m}") for m in range(2)]
                for t in range(2)
            ]
            hh = [hpool.tile([128, R], F16, tag=f"hh{t}", name=f"hh{t}") for t in range(2)]
            s1f = hpool.tile([128, R], F16, tag="s1f", name="s1f")
            sbst = [
                hpool.tile([128, 12 * NG], F32, tag=f"bst{s}", name=f"bst{s}") for s in range(4)
            ]
            tq = [hpool.tile([128, ncols], F16, tag=f"tq{q}", name=f"tq{q}") for q in range(4)]
            w2t = [hpool.tile([128, 256], F16, tag=f"w2t{t}", name=f"w2t{t}") for t in range(2)]
            bp = [hpool.tile([128, 2], F32, tag=f"bp{t}", name=f"bp{t}") for t in range(2)]
            arin = [hpool.tile([128, 4], F32, tag=f"arin{t}", name=f"arin{t}") for t in range(2)]
            agout = [hpool.tile([128, 32], F32, tag=f"agout{t}", name=f"agout{t}") for t in range(2)]
            fw = [hpool.tile([128, 32], F32, tag=f"fw{t}", name=f"fw{t}") for t in range(2)]
            stw = hpool.tile([128, 16], F32, tag="stw")
            scr = hpool.tile([1, 2], F32, tag="scr")

            bnc_in = [dpool.tile([128, 4], F32, name=f"bnc_in{t}") for t in range(2)]
            bnc_out = [
                dpool.tile([128 * N_CORES, 4], F32, addr_space="Shared", name=f"bnc_out{t}")
                for t in range(2)
            ]

            # preload the sqrt activation table set while DMAs warm up
            nc.scalar.activation(scr[0:1, 0:1], c32[0:1, ZERO_OFF : ZERO_OFF + 1], AF.Sqrt)

            # ================= phase 1 + per-tower stats + AllGathers =========
            with (
                tc.tile_pool(name="p1sb", bufs=3) as p1,
                tc.tile_pool(name="p1ps", bufs=2, space="PSUM") as pp1,
            ):
                for t in range(2):
                    for g in range(NG):
                        rows = slice(g * G, (g + 1) * G)
                        xtg = p1.tile([128, 4 * G], F16, tag="xtg")
                        nc.sync.dma_start(
                            xtg.rearrange("p (dc g) -> p dc g", dc=4),
                            xt_dram[t]
                            .rearrange("(dc p) r -> p dc r", p=128)[:, :, rows],
                        )
                        acc = [
                            pp1.tile([128, G], F32, tag=f"acc{m}", name=f"acc{m}") for m in range(2)
                        ]
                        for m in range(2):
                            for dc in range(4):
                                for sub in range(G // 512):
                                    nc.tensor.matmul(
                                        acc[m][:, sub * 512 : (sub + 1) * 512],
                                        w1s[:, dc * H + m * 128 : dc * H + (m + 1) * 128],
                                        xtg[:, dc * G + sub * 512 : dc * G + (sub + 1) * 512],
                                        start=(dc == 0),
                                        stop=(dc == 3),
                                    )
                        # evacuate h_pre to fp16; bn_stats on the fp16 copy
                        d0 = hp[t][0][:, rows]
                        nc.scalar.activation(d0, acc[0][:], AF.Copy)
                        d1 = hp[t][1][:, rows]
                        nc.vector.tensor_copy(d1, acc[1][:])
                        for m, dm in ((0, d0), (1, d1)):
                            for sub in range(2):
                                nc.vector.bn_stats(
                                    sbst[t * 2 + m][
                                        :, g * 12 + sub * 6 : g * 12 + (sub + 1) * 6
                                    ],
                                    dm[:, sub * 512 : (sub + 1) * 512],
                                )
                    # local stats for tower t -> arin[t]: cols 2m=mean, 2m+1=E[h^2]
                    for m in range(2):
                        s = t * 2 + m
                        aggr = stw[:, s * 4 : s * 4 + 2]
                        nc.vector.bn_aggr(aggr, sbst[s][:])
                        nc.vector.tensor_copy(arin[t][:, 2 * m : 2 * m + 1], aggr[:, 0:1])
                        msq = stw[:, s * 4 + 2 : s * 4 + 3]
                        nc.vector.tensor_tensor(msq, aggr[:, 0:1], aggr[:, 0:1], ALU.mult)
                        nc.vector.tensor_tensor(
                            arin[t][:, 2 * m + 1 : 2 * m + 2], aggr[:, 1:2], msq, ALU.add
                        )
                    nc.scalar.dma_start(bnc_in[t][:], arin[t][:])
                    nc.gpsimd.collective_compute(
                        "AllGather",
                        ALU.bypass,
                        ins=[bnc_in[t].opt()],
                        outs=[bnc_out[t].opt()],
                        replica_groups=[list(range(N_CORES))],
                    )

            # ---- fold + phases B/C (phase-1 pools closed) ----
            with (
                tc.tile_pool(name="p2sb", bufs=3) as p2,
                tc.tile_pool(name="p3sb", bufs=3) as p3,
                tc.tile_pool(name="p2ps", bufs=2, space="PSUM") as pp2,
                tc.tile_pool(name="p3ps", bufs=2, space="PSUM") as pp3,
            ):
                def fold(t):
                    nc.scalar.dma_start(
                        agout[t].rearrange("p (r j) -> p r j", r=N_CORES),
                        bnc_out[t].rearrange("(r p) j -> p r j", p=128),
                    )
                    w = fw[t]
                    nc.vector.tensor_tensor(
                        w[:, 0:16], agout[t][:, 0:16], agout[t][:, 16:32], ALU.add
                    )
                    nc.vector.tensor_tensor(w[:, 16:24], w[:, 0:8], w[:, 8:16], ALU.add)
                    nc.vector.tensor_tensor(
                        w[:, 24:28], w[:, 16:20], w[:, 20:24], ALU.add
                    )
                    mean2 = w[:, 28:30]
                    nc.vector.tensor_scalar(
                        mean2, w[:, 24:28:2], 1.0 / N_CORES, None, ALU.mult
                    )
                    eh2 = w[:, 30:32]
                    nc.vector.tensor_scalar(
                        eh2, w[:, 25:28:2], 1.0 / N_CORES, None, ALU.mult
                    )
                    nc.vector.tensor_tensor(w[:, 0:2], mean2, mean2, ALU.mult)
                    nc.vector.tensor_tensor(w[:, 2:4], eh2, w[:, 0:2], ALU.subtract)
                    nc.vector.tensor_scalar(
                        w[:, 4:6], w[:, 2:4], float(BN_EPS), None, ALU.add
                    )
                    nc.scalar.activation(w[:, 6:8], w[:, 4:6], AF.Sqrt)
                    nc.vector.reciprocal(w[:, 8:10], w[:, 6:8])
                    scale2 = w[:, 10:12]
                    nc.vector.tensor_tensor(scale2, w[:, 8:10], gammas, ALU.mult)
                    nc.vector.reciprocal(w[:, 12:14], scale2)
                    nc.vector.tensor_tensor(w[:, 14:16], betabns, w[:, 12:14], ALU.mult)
                    nc.vector.tensor_tensor(bp[t][:], w[:, 14:16], mean2, ALU.subtract)
                    for m in range(2):
                        nc.vector.tensor_scalar(
                            w2t[t][:, m * 128 : (m + 1) * 128],
                            w2s[:, m * 128 : (m + 1) * 128],
                            scale2[:, m : m + 1],
                            None, ALU.mult,
                        )

                def emit_B(t, c, act_evac):
                    cols = slice(c * CW, (c + 1) * CW)
                    hn0 = p2.tile([128, CW], F16, tag="hn0")
                    nc.vector.tensor_scalar(
                        hn0, hp[t][0][:, cols], bp[t][:, 0:1], zeros1, ALU.add, ALU.max
                    )
                    hn1 = p2.tile([128, CW], F16, tag="hn1")
                    nc.vector.tensor_scalar(
                        hn1, hp[t][1][:, cols], bp[t][:, 1:2], zeros1, ALU.add, ALU.max
                    )
                    for hf in range(2):
                        hs = slice(hf * 512, (hf + 1) * 512)
                        pw = pp2.tile([128, 512], F32, tag="pw")
                        nc.tensor.matmul(
                            pw[:], w2t[t][:, 0:128], hn0[:, hs], start=True, stop=False
                        )
                        nc.tensor.matmul(
                            pw[:], w2t[t][:, 128:256], hn1[:, hs], start=False, stop=True
                        )
                        dst = hh[t][:, c * CW + hf * 512 : c * CW + (hf + 1) * 512]
                        if act_evac and hf == 0:
                            nc.scalar.activation(dst, pw[:], AF.Relu, bias=b2s)
                        else:
                            nc.vector.tensor_scalar(
                                dst, pw[:], b2s, zeros1, ALU.add, ALU.max
                            )

                def emit_C(c):
                    cols = slice(c * CW, (c + 1) * CW)
                    h0 = hh[0][:, cols]
                    h1 = hh[1][:, cols]
                    # r on GpSimd (slow) -> emit first so it's ready for mm3
                    r_t = p3.tile([128, CW], F16, tag="r_t")
                    nc.gpsimd.tensor_tensor(r_t[:], h0, h1, ALU.add)
                    p_t = p3.tile([128, CW], F16, tag="p_t")
                    nc.vector.tensor_tensor(p_t[:], h0, h1, ALU.mult)
                    dd = p3.tile([128, CW], F16, tag="dd")
                    nc.vector.tensor_tensor(dd[:], h0, h1, ALU.subtract)
                    q_t = p3.tile([128, CW], F16, tag="q_t")
                    nc.scalar.activation(q_t[:], dd[:], AF.Abs)
                    s2_t = p3.tile([128, CW], F16, tag="s2_t")
                    nc.vector.tensor_tensor(s2_t[:], h1, h1, ALU.mult)

                    pw3 = pp3.tile([128, CW], F32, tag="pw3")
                    r67 = p3.tile([67, CW], F16, tag="r67")
                    for hf in range(2):
                        hs = slice(hf * 512, (hf + 1) * 512)
                        # chunk order matches W3 packing: [s1, p, q, r, s2]
                        rhs5 = [
                            s1f[:, c * CW + hf * 512 : c * CW + (hf + 1) * 512],
                            p_t[:, hs],
                            q_t[:, hs],
                            r_t[:, hs],
                            s2_t[:, hs],
                        ]
                        for k in range(5):
                            nc.tensor.matmul(
                                pw3[0:67, hs],
                                w3s[:, k * 67 : (k + 1) * 67],
                                rhs5[k],
                                start=(k == 0),
                                stop=(k == 4),
                            )
                        nc.scalar.activation(
                            r67[:, hs], pw3[0:67, hs], AF.Relu, bias=b3e
                        )
                    pw4 = pp3.tile([1, CW], F32, tag="pw4", bufs=1)
                    for hf in range(2):
                        hs = slice(hf * 512, (hf + 1) * 512)
                        nc.tensor.matmul(
                            pw4[:, hs], w4s, r67[0:64, hs], start=True, stop=True
                        )
                    stg = p3.tile([1, CW], F16, tag="stg")
                    nc.vector.tensor_copy(stg[:], pw4[:])
                    for i in range(3):
                        nc.scalar.dma_start(
                            tq[i][c * ppc : (c + 1) * ppc, :], r67[64 + i : 65 + i, :]
                        )
                    nc.scalar.dma_start(tq[3][c * ppc : (c + 1) * ppc, :], stg[:])

                # tower 1: fold + B + s1 precompute (overlaps AllGather #2)
                fold(0)
                for c in range(NB):
                    emit_B(0, c, act_evac=True)
                    cols = slice(c * CW, (c + 1) * CW)
                    nc.vector.tensor_tensor(
                        s1f[:, cols], hh[0][:, cols], hh[0][:, cols], ALU.mult
                    )
                # tower 2 + combine
                fold(1)
                for c in range(NB):
                    emit_B(1, c, act_evac=False)
                    emit_C(c)

            # ================= phase D: finalize =================
            fin = hpool.tile([128, 6 * ncols], F32, tag="fin")

            def fcol(i):
                return fin[:, i * ncols : (i + 1) * ncols]

            nc.vector.tensor_tensor(fcol(0), tq[1][:], tq[2][:], ALU.mult)  # n1*n2
            nc.vector.tensor_scalar(fcol(2), fcol(0), 1e-30, None, ALU.add)
            nc.scalar.activation(fcol(1), fcol(2), AF.Sqrt)
            nc.vector.reciprocal(fcol(2), fcol(1))
            nc.vector.tensor_tensor(fcol(0), tq[0][:], fcol(2), ALU.mult)  # s_math
            nc.vector.tensor_scalar(fcol(1), fcol(0), 0.0, 1.0, ALU.max, ALU.min)
            nc.scalar.activation(fcol(3), tq[3][:], AF.Sigmoid, bias=b4s)
            nc.vector.tensor_scalar(fcol(4), fcol(1), alphas, None, ALU.mult)
            nc.vector.tensor_scalar(fcol(5), fcol(3), betas, None, ALU.mult)
            nc.vector.tensor_tensor(fcol(0), fcol(4), fcol(5), ALU.add)
            nc.vector.tensor_scalar(fcol(1), fcol(0), 0.0, 1.0, ALU.max, ALU.min)
            nc.sync.dma_start(
                out_d.ap().rearrange("(p k) -> p k", p=128), fcol(1)
            )

    nc.compile()
    return nc


_NC_CACHE: dict = {}


def _get_nc(n_chunks):
    if n_chunks not in _NC_CACHE:
        _NC_CACHE[n_chunks] = build_nc(n_chunks)
    return _NC_CACHE[n_chunks]


def _prep_weights(W1, gamma, beta_bn, W2, b2, W3, b3, W4, b4, alpha, beta):
    f16 = np.float16
    f32 = np.float32
    W1 = np.asarray(W1, f32)
    W3 = np.asarray(W3, f32)
    w3e = np.zeros((128, 5, 67), f32)
    # chunk order matches rhs5 = [s1, p, q, r, s2]
    w3e[:, 1, 0:64] = W3[0:128]    # p  (h1*h2)
    w3e[:, 2, 0:64] = W3[128:256]  # q  (|h1-h2|)
    w3e[:, 3, 0:64] = W3[256:384]  # r  (h1+h2)
    w3e[:, 1, 64] = 1.0  # dot = ones . (h1*h2)
    w3e[:, 0, 65] = 1.0  # n1  = ones . h1^2
    w3e[:, 4, 66] = 1.0  # n2  = ones . h2^2
    c16 = np.zeros((128, F16_COLS), f16)
    c16[:, W1_OFF : W1_OFF + 1024] = (
        W1.reshape(4, 128, H).transpose(1, 0, 2).reshape(128, 1024).astype(f16)
    )
    c16[:, W2_OFF : W2_OFF + 256] = (
        np.asarray(W2, f32).reshape(2, 128, 128).transpose(1, 0, 2).reshape(128, 256)
    ).astype(f16)
    c16[:, W3_OFF : W3_OFF + 335] = w3e.reshape(128, 335).astype(f16)
    c16[0:64, W4_OFF] = np.asarray(W4, f32).reshape(-1).astype(f16)
    c32 = np.zeros((128, F32_COLS), f32)
    c32[:, B2_OFF] = np.asarray(b2, f32).reshape(-1)
    c32[:, GAM_OFF : GAM_OFF + 2] = np.asarray(gamma, f32).reshape(2, 128).T
    c32[:, BBN_OFF : BBN_OFF + 2] = np.asarray(beta_bn, f32).reshape(2, 128).T
    c32[:, ALP_OFF] = np.asarray(alpha, f32).reshape(-1)[0]
    c32[:, BET_OFF] = np.asarray(beta, f32).reshape(-1)[0]
    c32[:, B4_OFF] = np.asarray(b4, f32).reshape(-1)[0]
    c32[0:64, B3_OFF] = np.asarray(b3, f32).reshape(-1)
    return {"c16": c16, "c32": c32}


def run_on_hw(x1, x2, weights, n_chunks, trace=False):
    R = n_chunks * 512
    nc = _get_nc(n_chunks)
    x1h = np.asarray(x1, np.float32).astype(np.float16)
    x2h = np.asarray(x2, np.float32).astype(np.float16)
    in_maps = []
    for c in range(N_CORES):
        m = {
            "xt1": np.ascontiguousarray(x1h[c * R : (c + 1) * R].T),
            "xt2": np.ascontiguousarray(x2h[c * R : (c + 1) * R].T),
        }
        m.update(weights)
        in_maps.append(m)
    r = bass_utils.run_bass_kernel_spmd(
        nc, in_maps, core_ids=list(range(N_CORES)), trace=trace
    )
    outs = [r.results[c]["out"].reshape(-1) for c in range(N_CORES)]
    return np.concatenate(outs), r


def kernel(x1, x2, W1, b1, gamma, beta_bn, W2, b2, W3, b3, W4, b4, alpha, beta):
    x1 = np.asarray(x1, np.float32)
    x2 = np.asarray(x2, np.float32)
    n_chunks = x1.shape[0] // (N_CORES * 512)
    weights = _prep_weights(W1, gamma, beta_bn, W2, b2, W3, b3, W4, b4, alpha, beta)
    out, _ = run_on_hw(x1, x2, weights, n_chunks)
    return out.astype(np.float32)
